# revision 43
# baseline (speedup 1.0000x reference)
"""Multi-head attention (B=2, S=2048, D=1024, H=16) on 8 TRN2 NeuronCores.

Sharding: tensor-parallel over heads x data-parallel over batch.
Core c handles batch b = c//4, head group g = c%4 (4 heads, 256 cols).
W_q/W_k/W_v are split column-wise per group, W_o row-wise; each core
produces a partial [S, D] output, reduced on the host (the W_o
contraction is a pure sum over head groups; b_v/b_o folded in on host).

Device kernel (per core), all matmuls bf16 with fp32 PSUM accumulation:
  - K^T, Q^T projections in transposed layout [dk*2, S] (lhsT = W cols,
    rhs = x^T), V in natural layout [S, dk*4+ones] (lhsT = x^T chunks).
  - scores computed transposed: ST[k,q] = (K^T chunk)^T-matmul vs Q^T,
    softmax without max-subtraction (logits are O(5) here, exp is safe):
    exp on ACT straight out of PSUM with scale=1/sqrt(dk).
  - ctx^T[dk+1, q] accumulated over k-chunks with an ones-augmented V
    (row dk = softmax denominators), normalized via DVE with a gpsimd
    partition-broadcast of the reciprocals.
  - out partial = ctx^T-chunks @ W_o rows, accumulated over the 2
    128-row chunks of the group's 256 W_o rows.
"""

import numpy as np
import ml_dtypes
from contextlib import ExitStack

import concourse.bass as bass
import concourse.tile as tile
from concourse import bacc, mybir
from concourse.bass_utils import run_bass_kernel_spmd

BF16 = mybir.dt.bfloat16
F32 = mybir.dt.float32

D = 1024            # model dim
H = 16              # heads
DK = 64             # head dim
NCORES = 8
GPB = 4             # head groups per batch (= cores per batch)
HPG = H // GPB      # 4 heads per core
HD = HPG * DK       # 256 cols per group
HAUG = DK + 1       # 65: head block width in augmented-V layout
SP = 512            # q-span / free-dim tile
SCALE = 1.0 / np.sqrt(DK)


def build(S, debug_dump=False):
    NQS = S // SP       # q spans
    NSC = S // 128      # sequence chunks (k side)
    NDC = D // 128      # model-dim chunks
    SI = SP // 128      # s-chunks per q-span

    nc = bacc.Bacc("TRN2", target_bir_lowering=False, debug=False)
    if debug_dump:
        kt_d = nc.dram_tensor("kt_d", [128, 2, S], BF16, kind="ExternalOutput")
        v_d = nc.dram_tensor("v_d", [128, NSC, HPG * HAUG], BF16,
                             kind="ExternalOutput")
        qt_d = nc.dram_tensor("qt_d", [128, 2, SP], BF16, kind="ExternalOutput")
        ct_d = nc.dram_tensor("ct_d", [128, 2, SP], BF16, kind="ExternalOutput")
    HA = HPG * HAUG     # 260: augmented V width
    xT_e = nc.dram_tensor("xT", [S // SP, 128, D // 128, SP], BF16, kind="ExternalInput")
    wq_e = nc.dram_tensor("wq", [128, D // 128, HD], BF16, kind="ExternalInput")
    wk_e = nc.dram_tensor("wk", [128, D // 128, HD], BF16, kind="ExternalInput")
    wv_e = nc.dram_tensor("wv", [128, D // 128, HA], BF16, kind="ExternalInput")
    vmask_e = nc.dram_tensor("vmask", [1, HA], BF16, kind="ExternalInput")
    wo_e = nc.dram_tensor("wo", [128, 2, D], BF16, kind="ExternalInput")
    bq_e = nc.dram_tensor("bq", [128, 2], F32, kind="ExternalInput")
    bk_e = nc.dram_tensor("bk", [128, 2], F32, kind="ExternalInput")
    ones_e = nc.dram_tensor("ones", [128, HPG], BF16, kind="ExternalInput")
    out_e = nc.dram_tensor("out", [S, D], F32, kind="ExternalOutput")

    ADD = mybir.AluOpType.add
    MULT = mybir.AluOpType.mult
    EXP = mybir.ActivationFunctionType.Exp

    with tile.TileContext(nc) as tc, ExitStack() as ctx:
        const = ctx.enter_context(tc.tile_pool(name="const", bufs=1))
        qpool = ctx.enter_context(tc.tile_pool(name="qpool", bufs=2))
        cpool = ctx.enter_context(tc.tile_pool(name="cpool", bufs=2))
        ptp = ctx.enter_context(tc.tile_pool(name="ptp", bufs=6))
        obp = ctx.enter_context(tc.tile_pool(name="obp", bufs=4))
        smp = ctx.enter_context(tc.tile_pool(name="smp", bufs=3))
        psum = ctx.enter_context(tc.tile_pool(name="psum", bufs=2, space="PSUM"))

        wq_sb = const.tile([128, NDC, HD], BF16, name="wq_sb")
        wk_sb = const.tile([128, NDC, HD], BF16, name="wk_sb")
        wv_sb = const.tile([128, NDC, HA], BF16, name="wv_sb")
        vmask_sb = const.tile([1, HA], BF16, name="vmask_sb")
        ones_col = const.tile([1, 128], BF16, name="ones_col")
        wo_sb = const.tile([128, 2, D], BF16, name="wo_sb")
        bq_sb = const.tile([128, 2], F32, name="bq_sb")
        bk_sb = const.tile([128, 2], F32, name="bk_sb")
        xT_sb = [const.tile([128, NDC, SP], BF16, name=f"xT{q}") for q in range(NQS)]
        KT_sb = const.tile([128, 2, S], BF16, name="KT_sb")
        V_sb = const.tile([128, NSC, HPG * HAUG], BF16, name="V_sb")

        # input DMAs: host pre-tiles everything to the exact SBUF layout,
        # so each tensor is one flat contiguous transfer.
        nc.sync.dma_start(wk_sb[:, 0:2, :], wk_e.ap()[:, 0:2, :])
        nc.sync.dma_start(bk_sb[:], bk_e.ap())
        nc.sync.dma_start(xT_sb[0][:, 0:2, :], xT_e.ap()[0, :, 0:2, :])
        nc.sync.dma_start(wk_sb[:, 2:, :], wk_e.ap()[:, 2:, :])
        nc.sync.dma_start(xT_sb[0][:, 2:, :], xT_e.ap()[0, :, 2:, :])
        nc.sync.dma_start(wv_sb[:], wv_e.ap())
        nc.sync.dma_start(vmask_sb[:], vmask_e.ap())
        nc.sync.dma_start(ones_col[:], ones_e.ap()[0:32, :])
        nc.sync.dma_start(wq_sb[:], wq_e.ap())
        nc.sync.dma_start(bq_sb[:], bq_e.ap())
        for q in range(1, NQS):
            nc.sync.dma_start(xT_sb[q][:], xT_e.ap()[q])
        nc.sync.dma_start(wo_sb[:], wo_e.ap())

        # K^T projection group: KT[128 (2 heads), m, s]
        def emit_kproj_group(m, q, tag="mm"):
            ps = psum.tile([128, SP], F32, tag=tag, name="kps")
            for c in range(NDC):
                nc.tensor.matmul(
                    ps[:], wk_sb[:, c, m * 128:(m + 1) * 128],
                    xT_sb[q][:, c, :],
                    start=(c == 0), stop=(c == NDC - 1))
            nc.vector.tensor_scalar(
                KT_sb[:, m, q * SP:(q + 1) * SP], ps[:],
                bk_sb[:, m:m + 1], None, ADD)

        # V projection into augmented layout [s-chunk, 4*(64+1)];
        # the ones columns come from a K=1 outer product with vmask
        def emit_vproj_group(sc, tag="mm"):
            q, si = divmod(sc, SI)
            ps = psum.tile([128, HA], F32, tag=tag, name="vps")
            for c in range(NDC):
                nc.tensor.matmul(
                    ps[:], xT_sb[q][:, c, si * 128:(si + 1) * 128],
                    wv_sb[:, c, :],
                    start=(c == 0), stop=False)
            nc.tensor.matmul(ps[:], ones_col[:], vmask_sb[:],
                             start=False, stop=True)
            nc.vector.tensor_copy(V_sb[:, sc, :], ps[:])

        # prologue: only what span-0 attention needs immediately — K^T
        # for heads 0/1, the first half of V. The rest (K^T m=1, V 8..15,
        # Q^T m=1) interleaves into span 0's attention stream below.
        NV_PRE = min(8, NSC)
        for q in range(NQS):
            emit_kproj_group(0, q)
        for sc in range(NV_PRE):
            emit_vproj_group(sc)

        # ---- per-span machinery -------------------------------------
        # Attention is software-pipelined: ctx matmuls lag one (h, scp)
        # group behind logits+exp so the in-order PE queue never blocks
        # on the ACT exp of the current group. Dense matmul work (the
        # previous span's W_o and the next span's Q^T projection) is
        # interleaved into the attention stream to keep PE streaming
        # duty high enough that the HAM clock gate stays at full rate.

        def emit_qproj_group(QTn, qsrc, m):
            ps = psum.tile([128, SP], F32, tag="wo", name="qps")
            for c in range(NDC):
                nc.tensor.matmul(
                    ps[:], wq_sb[:, c, m * 128:(m + 1) * 128],
                    xT_sb[qsrc][:, c, :],
                    start=(c == 0), stop=(c == NDC - 1))
            nc.vector.tensor_scalar(
                QTn[:, m, :], ps[:], bq_sb[:, m:m + 1], None, ADD)

        def emit_lg_exp(QT, h, scp):
            m, r = divmod(h, 2)
            r *= 64
            lg = psum.tile([128, 2 * SP], F32, tag="mm", name="lg")
            for j in range(2):
                sc = 2 * scp + j
                nc.tensor.matmul(
                    lg[:, j * SP:(j + 1) * SP],
                    KT_sb[r:r + 64, m, sc * 128:(sc + 1) * 128],
                    QT[r:r + 64, m, :],
                    start=True, stop=True)
            pt = ptp.tile([128, 2 * SP], BF16, name="pt")
            nc.scalar.activation(pt[:], lg[:], EXP, scale=float(SCALE))
            return pt

        def emit_ctx(CT, cps_by_h, h, scp, pt):
            if scp == 0:
                cps_by_h[h] = psum.tile([HAUG, SP], F32, tag="ctx",
                                        name="cps")
            cps = cps_by_h[h]
            for j in range(2):
                sc = 2 * scp + j
                nc.tensor.matmul(
                    cps[:], V_sb[:, sc, h * HAUG:(h + 1) * HAUG],
                    pt[:, j * SP:(j + 1) * SP],
                    start=(sc == 0), stop=(sc == NSC - 1))
            if scp == NSC // 2 - 1:
                return emit_norm(CT, h, cps)
            return None

        def emit_norm(CT, h, cps):
            # deferred: runs a few jobs later so nothing here sits at the
            # head of the PE queue. No PE instruction in this chain —
            # the partition broadcast runs on the (otherwise idle) gpsimd.
            def run():
                m, r = divmod(h, 2)
                r *= 64
                sm = smp.tile([1, SP], F32, name="sm")
                nc.vector.tensor_copy(sm[:], cps[DK:DK + 1, :])
                rc = smp.tile([1, SP], F32, name="rc")
                nc.vector.reciprocal_approx_fast(rc[:], sm[:])
                bc = smp.tile([64, SP], F32, name="bc")
                nc.gpsimd.partition_broadcast(bc[:], rc[:])
                nc.vector.tensor_tensor(
                    CT[r:r + 64, m, :], cps[0:DK, :], bc[:], MULT)
            return run

        def make_wo_ops(q, CT, split_copies=False):
            ops = []
            for si in range(SI):
                sc = SI * q + si
                for dh in range(D // SP):
                    on_act = split_copies and (si * (D // SP) + dh) % 2 == 1
                    def op(si=si, sc=sc, dh=dh, CT=CT, on_act=on_act):
                        po = psum.tile([128, SP], F32, tag="wo", name="po")
                        for m in range(2):
                            nc.tensor.matmul(
                                po[:], CT[:, m, si * 128:(si + 1) * 128],
                                wo_sb[:, m, dh * SP:(dh + 1) * SP],
                                start=(m == 0), stop=(m == 1))
                        ob = obp.tile([128, SP], F32, name="ob")
                        if on_act:
                            # tail only: ACT is idle once the exps are done
                            nc.scalar.activation(
                                ob[:], po[:],
                                mybir.ActivationFunctionType.Copy)
                        else:
                            nc.vector.tensor_copy(ob[:], po[:])
                        nc.sync.dma_start(
                            out_e.ap()[sc * 128:(sc + 1) * 128,
                                       dh * SP:(dh + 1) * SP], ob[:])
                    ops.append(op)
            return ops

        # One flat pipeline across all spans: ctx lags one job, norms
        # run two jobs late, the previous span's W_o and the next span's
        # Q^T projection interleave — nothing flushes at span edges
        # except at the very end of the kernel.
        JPS = HPG * (NSC // 2)          # jobs per span
        QT_t = {0: qpool.tile([128, 2, SP], BF16, name="QT")}
        emit_qproj_group(QT_t[0], 0, 0)
        CT_t = {}
        cps_t = {}
        LAG = 2
        pend_q = []                     # [(q, h, scp, pt)]
        deferred = []                   # (due_gidx, closure)
        wo_queue = []
        # deferred prologue work, one op per early span-0 job, on wo-tag
        # psum slots (no W_o traffic exists yet in span 0)
        fill_queue = [lambda sc=sc: emit_vproj_group(sc, tag="wo")
                      for sc in range(NV_PRE, NSC)]
        fill_queue += [lambda q=q: emit_kproj_group(1, q, tag="wo")
                       for q in range(NQS)]
        fill_queue.append(lambda: emit_qproj_group(QT_t[0], 0, 1))

        for q in range(NQS):
            CT_t[q] = cpool.tile([128, 2, SP], BF16, name="CT")
            cps_t[q] = {}
            if q + 1 < NQS:
                QT_t[q + 1] = qpool.tile([128, 2, SP], BF16, name="QT")

            for jidx in range(JPS):
                gidx = q * JPS + jidx
                h, scp = divmod(jidx, NSC // 2)
                pt = emit_lg_exp(QT_t[q], h, scp)
                pend_q.append((q, h, scp, pt))
                if len(pend_q) > LAG:
                    ent0 = pend_q.pop(0)
                    nrm = emit_ctx(CT_t[ent0[0]], cps_t[ent0[0]], *ent0[1:])
                    if nrm is not None:
                        deferred.append((gidx + 2, nrm))
                for ent in list(deferred):
                    if ent[0] <= gidx:
                        ent[1]()
                        deferred.remove(ent)
                if fill_queue:
                    fill_queue.pop(0)()
                if wo_queue and jidx >= 6 and (jidx - 6) % 3 == 0:
                    wo_queue.pop(0)()
                if q + 1 < NQS and jidx in (14, 22):
                    emit_qproj_group(QT_t[q + 1], q + 1, (jidx - 14) // 8)
            wo_queue.extend(make_wo_ops(q, CT_t[q],
                                           split_copies=(q == NQS - 1)))

        # epilogue: drain the pipeline
        last_nrm = None
        for ent0 in pend_q:
            nrm = emit_ctx(CT_t[ent0[0]], cps_t[ent0[0]], *ent0[1:])
            if nrm is not None:
                last_nrm = nrm
        for ent in deferred:
            ent[1]()
        if last_nrm is not None:
            last_nrm()
        for op in wo_queue:
            op()

        if debug_dump:
            nc.sync.dma_start(ct_d.ap(), CT_t[NQS - 1][:])
            nc.sync.dma_start(kt_d.ap(), KT_sb[:])
            nc.sync.dma_start(v_d.ap(), V_sb[:])

    nc.compile()
    return nc


_NC_CACHE = {}


def get_nc(S):
    if S not in _NC_CACHE:
        _NC_CACHE[S] = build(S)
    return _NC_CACHE[S]


def make_in_maps(x, W_q, b_q, W_k, b_k, W_v, b_v, W_o, b_o):
    B, S, _ = x.shape
    bf = ml_dtypes.bfloat16
    in_maps = []
    vmask = np.zeros((1, HPG * HAUG), np.float32)
    vmask[0, DK::HAUG] = 1.0
    for core in range(NCORES):
        b, g = divmod(core, GPB)
        sl = slice(HD * g, HD * (g + 1))
        wv_aug = np.zeros((D, HPG * HAUG), np.float32)
        wv_block = np.asarray(W_v[:, sl]).reshape(D, HPG, DK)
        wv_aug.reshape(D, HPG, HAUG)[:, :, :DK] = wv_block
        def wtile(w):
            # [D, N] -> [128, D//128, N] partition-major chunk layout
            return np.ascontiguousarray(
                np.asarray(w).reshape(D // 128, 128, -1).transpose(1, 0, 2))
        in_maps.append({
            "xT": np.ascontiguousarray(
                np.asarray(x[b]).T.reshape(D // 128, 128, S // SP, SP)
                .transpose(2, 1, 0, 3)).astype(bf),
            "wq": wtile(W_q[:, sl]).astype(bf),
            "wk": wtile(W_k[:, sl]).astype(bf),
            "wv": wtile(wv_aug).astype(bf),
            "vmask": vmask.astype(bf),
            "wo": np.ascontiguousarray(
                np.asarray(W_o[sl, :]).reshape(2, 128, D)
                .transpose(1, 0, 2)).astype(bf),
            "bq": np.ascontiguousarray(
                np.asarray(b_q[sl]).reshape(2, 128).T).astype(np.float32),
            "bk": np.ascontiguousarray(
                np.asarray(b_k[sl]).reshape(2, 128).T).astype(np.float32),
            "ones": np.ones((128, HPG), dtype=bf),
        })
    return in_maps


def unshard(results, x, W_o, b_v, b_o):
    B, S, _ = x.shape
    out = np.zeros((B, S, D), np.float32)
    for core in range(NCORES):
        b = core // GPB
        out[b] += results[core]["out"]
    const = np.asarray(b_v).astype(np.float64) @ np.asarray(W_o).astype(np.float64)
    const += np.asarray(b_o).astype(np.float64)
    out += const.astype(np.float32)[None, None, :]
    return out


def run(inputs, trace=False):
    x = np.asarray(inputs["x"])
    nc = get_nc(x.shape[1])
    in_maps = make_in_maps(
        x, inputs["W_q"], inputs["b_q"], inputs["W_k"], inputs["b_k"],
        inputs["W_v"], inputs["b_v"], inputs["W_o"], inputs["b_o"])
    try:
        res = run_bass_kernel_spmd(
            nc, in_maps, core_ids=list(range(NCORES)), trace=trace)
    except Exception:
        # transient device errors (e.g. NRT_EXEC_UNIT_UNRECOVERABLE) clear
        # on re-execution of the same NEFF
        res = run_bass_kernel_spmd(
            nc, in_maps, core_ids=list(range(NCORES)), trace=trace)
    out = unshard(res.results, x, inputs["W_o"], inputs["b_v"], inputs["b_o"])
    return out, res


def kernel(**inputs):
    out, _ = run(inputs, trace=False)
    return out


# revision 44
# speedup vs baseline: 1.0088x; 1.0088x over previous
"""Multi-head attention (B=2, S=2048, D=1024, H=16) on 8 TRN2 NeuronCores.

Sharding: tensor-parallel over heads x data-parallel over batch.
Core c handles batch b = c//4, head group g = c%4 (4 heads, 256 cols).
W_q/W_k/W_v are split column-wise per group, W_o row-wise; each core
produces a partial [S, D] output, reduced on the host (the W_o
contraction is a pure sum over head groups; b_v/b_o folded in on host).

Device kernel (per core), all matmuls bf16 with fp32 PSUM accumulation:
  - K^T, Q^T projections in transposed layout [dk*2, S] (lhsT = W cols,
    rhs = x^T), V in natural layout [S, dk*4+ones] (lhsT = x^T chunks).
  - scores computed transposed: ST[k,q] = (K^T chunk)^T-matmul vs Q^T,
    softmax without max-subtraction (logits are O(5) here, exp is safe):
    exp on ACT straight out of PSUM with scale=1/sqrt(dk).
  - ctx^T[dk+1, q] accumulated over k-chunks with an ones-augmented V
    (row dk = softmax denominators), normalized via DVE with a gpsimd
    partition-broadcast of the reciprocals.
  - out partial = ctx^T-chunks @ W_o rows, accumulated over the 2
    128-row chunks of the group's 256 W_o rows.
"""

import numpy as np
import ml_dtypes
from contextlib import ExitStack

import concourse.bass as bass
import concourse.tile as tile
from concourse import bacc, mybir
from concourse.bass_utils import run_bass_kernel_spmd

BF16 = mybir.dt.bfloat16
F32 = mybir.dt.float32

D = 1024            # model dim
H = 16              # heads
DK = 64             # head dim
NCORES = 8
GPB = 4             # head groups per batch (= cores per batch)
HPG = H // GPB      # 4 heads per core
HD = HPG * DK       # 256 cols per group
HAUG = DK + 1       # 65: head block width in augmented-V layout
SP = 512            # q-span / free-dim tile
SCALE = 1.0 / np.sqrt(DK)


def build(S, debug_dump=False):
    NQS = S // SP       # q spans
    NSC = S // 128      # sequence chunks (k side)
    NDC = D // 128      # model-dim chunks
    SI = SP // 128      # s-chunks per q-span

    nc = bacc.Bacc("TRN2", target_bir_lowering=False, debug=False)
    if debug_dump:
        kt_d = nc.dram_tensor("kt_d", [128, 2, S], BF16, kind="ExternalOutput")
        v_d = nc.dram_tensor("v_d", [128, NSC, HPG * HAUG], BF16,
                             kind="ExternalOutput")
        qt_d = nc.dram_tensor("qt_d", [128, 2, SP], BF16, kind="ExternalOutput")
        ct_d = nc.dram_tensor("ct_d", [128, 2, SP], BF16, kind="ExternalOutput")
    HA = HPG * HAUG     # 260: augmented V width
    xT_e = nc.dram_tensor("xT", [S // SP, 128, D // 128, SP], BF16, kind="ExternalInput")
    wq_e = nc.dram_tensor("wq", [128, D // 128, HD], BF16, kind="ExternalInput")
    wk_e = nc.dram_tensor("wk", [128, D // 128, HD], BF16, kind="ExternalInput")
    wv_e = nc.dram_tensor("wv", [128, D // 128, HA], BF16, kind="ExternalInput")
    vmask_e = nc.dram_tensor("vmask", [1, HA], BF16, kind="ExternalInput")
    wo_e = nc.dram_tensor("wo", [128, 2, D], BF16, kind="ExternalInput")
    bq_e = nc.dram_tensor("bq", [128, 2], F32, kind="ExternalInput")
    bk_e = nc.dram_tensor("bk", [128, 2], F32, kind="ExternalInput")
    ones_e = nc.dram_tensor("ones", [128, HPG], BF16, kind="ExternalInput")
    out_e = nc.dram_tensor("out", [S, D], F32, kind="ExternalOutput")

    ADD = mybir.AluOpType.add
    MULT = mybir.AluOpType.mult
    EXP = mybir.ActivationFunctionType.Exp

    with tile.TileContext(nc) as tc, ExitStack() as ctx:
        const = ctx.enter_context(tc.tile_pool(name="const", bufs=1))
        qpool = ctx.enter_context(tc.tile_pool(name="qpool", bufs=2))
        cpool = ctx.enter_context(tc.tile_pool(name="cpool", bufs=2))
        ptp = ctx.enter_context(tc.tile_pool(name="ptp", bufs=6))
        obp = ctx.enter_context(tc.tile_pool(name="obp", bufs=4))
        smp = ctx.enter_context(tc.tile_pool(name="smp", bufs=3))
        psum = ctx.enter_context(tc.tile_pool(name="psum", bufs=2, space="PSUM"))

        wq_sb = const.tile([128, NDC, HD], BF16, name="wq_sb")
        wk_sb = const.tile([128, NDC, HD], BF16, name="wk_sb")
        wv_sb = const.tile([128, NDC, HA], BF16, name="wv_sb")
        vmask_sb = const.tile([1, HA], BF16, name="vmask_sb")
        ones_col = const.tile([1, 128], BF16, name="ones_col")
        wo_sb = const.tile([128, 2, D], BF16, name="wo_sb")
        bq_sb = const.tile([128, 2], F32, name="bq_sb")
        bk_sb = const.tile([128, 2], F32, name="bk_sb")
        xT_sb = [const.tile([128, NDC, SP], BF16, name=f"xT{q}") for q in range(NQS)]
        KT_sb = const.tile([128, 2, S], BF16, name="KT_sb")
        V_sb = const.tile([128, NSC, HPG * HAUG], BF16, name="V_sb")

        # input DMAs: host pre-tiles everything to the exact SBUF layout,
        # so each tensor is one flat contiguous transfer.
        nc.sync.dma_start(wk_sb[:, 0:2, :], wk_e.ap()[:, 0:2, :])
        nc.sync.dma_start(bk_sb[:], bk_e.ap())
        nc.sync.dma_start(xT_sb[0][:, 0:2, :], xT_e.ap()[0, :, 0:2, :])
        nc.sync.dma_start(wk_sb[:, 2:, :], wk_e.ap()[:, 2:, :])
        nc.sync.dma_start(xT_sb[0][:, 2:, :], xT_e.ap()[0, :, 2:, :])
        nc.sync.dma_start(wv_sb[:], wv_e.ap())
        nc.sync.dma_start(vmask_sb[:], vmask_e.ap())
        nc.sync.dma_start(ones_col[:], ones_e.ap()[0:32, :])
        nc.sync.dma_start(wq_sb[:], wq_e.ap())
        nc.sync.dma_start(bq_sb[:], bq_e.ap())
        for q in range(1, NQS):
            nc.sync.dma_start(xT_sb[q][:], xT_e.ap()[q])
        nc.sync.dma_start(wo_sb[:], wo_e.ap())

        # K^T projection group: KT[128 (2 heads), m, s]
        def emit_kproj_group(m, q, tag="mm"):
            ps = psum.tile([128, SP], F32, tag=tag, name="kps")
            for c in range(NDC):
                nc.tensor.matmul(
                    ps[:], wk_sb[:, c, m * 128:(m + 1) * 128],
                    xT_sb[q][:, c, :],
                    start=(c == 0), stop=(c == NDC - 1))
            nc.vector.tensor_scalar(
                KT_sb[:, m, q * SP:(q + 1) * SP], ps[:],
                bk_sb[:, m:m + 1], None, ADD)

        # V projection into augmented layout [s-chunk, 4*(64+1)];
        # the ones columns come from a K=1 outer product with vmask
        def emit_vproj_group(sc, tag="mm"):
            q, si = divmod(sc, SI)
            ps = psum.tile([128, HA], F32, tag=tag, name="vps")
            for c in range(NDC):
                nc.tensor.matmul(
                    ps[:], xT_sb[q][:, c, si * 128:(si + 1) * 128],
                    wv_sb[:, c, :],
                    start=(c == 0), stop=False)
            nc.tensor.matmul(ps[:], ones_col[:], vmask_sb[:],
                             start=False, stop=True)
            nc.vector.tensor_copy(V_sb[:, sc, :], ps[:])

        # prologue: only what span-0 attention needs immediately — K^T
        # for heads 0/1, the first half of V. The rest (K^T m=1, V 8..15,
        # Q^T m=1) interleaves into span 0's attention stream below.
        NV_PRE = min(8, NSC)
        for q in range(NQS):
            emit_kproj_group(0, q)
        for sc in range(NV_PRE):
            emit_vproj_group(sc)

        # ---- per-span machinery -------------------------------------
        # Attention is software-pipelined: ctx matmuls lag one (h, scp)
        # group behind logits+exp so the in-order PE queue never blocks
        # on the ACT exp of the current group. Dense matmul work (the
        # previous span's W_o and the next span's Q^T projection) is
        # interleaved into the attention stream to keep PE streaming
        # duty high enough that the HAM clock gate stays at full rate.

        def make_qproj_parts(QTn, qsrc, m, nparts=2):
            cell = []
            step = NDC // nparts
            def part(p):
                def run():
                    if p == 0:
                        cell.append(psum.tile([128, SP], F32, tag="wo",
                                              name="qps"))
                    ps = cell[0]
                    for c in range(p * step, (p + 1) * step):
                        nc.tensor.matmul(
                            ps[:], wq_sb[:, c, m * 128:(m + 1) * 128],
                            xT_sb[qsrc][:, c, :],
                            start=(c == 0), stop=(c == NDC - 1))
                    if p == nparts - 1:
                        nc.vector.tensor_scalar(
                            QTn[:, m, :], ps[:], bq_sb[:, m:m + 1], None, ADD)
                return run
            return [part(p) for p in range(nparts)]

        def emit_qproj_group(QTn, qsrc, m):
            for run in make_qproj_parts(QTn, qsrc, m, nparts=1):
                run()

        def emit_lg_exp(QT, h, scp):
            m, r = divmod(h, 2)
            r *= 64
            lg = psum.tile([128, 2 * SP], F32, tag="mm", name="lg")
            for j in range(2):
                sc = 2 * scp + j
                nc.tensor.matmul(
                    lg[:, j * SP:(j + 1) * SP],
                    KT_sb[r:r + 64, m, sc * 128:(sc + 1) * 128],
                    QT[r:r + 64, m, :],
                    start=True, stop=True)
            pt = ptp.tile([128, 2 * SP], BF16, name="pt")
            nc.scalar.activation(pt[:], lg[:], EXP, scale=float(SCALE))
            return pt

        def emit_ctx(CT, cps_by_h, h, scp, pt):
            if scp == 0:
                cps_by_h[h] = psum.tile([HAUG, SP], F32, tag="ctx",
                                        name="cps")
            cps = cps_by_h[h]
            for j in range(2):
                sc = 2 * scp + j
                nc.tensor.matmul(
                    cps[:], V_sb[:, sc, h * HAUG:(h + 1) * HAUG],
                    pt[:, j * SP:(j + 1) * SP],
                    start=(sc == 0), stop=(sc == NSC - 1))
            if scp == NSC // 2 - 1:
                return emit_norm(CT, h, cps)
            return None

        def emit_norm(CT, h, cps):
            # deferred: runs a few jobs later so nothing here sits at the
            # head of the PE queue. No PE instruction in this chain —
            # the partition broadcast runs on the (otherwise idle) gpsimd.
            def run():
                m, r = divmod(h, 2)
                r *= 64
                sm = smp.tile([1, SP], F32, name="sm")
                nc.vector.tensor_copy(sm[:], cps[DK:DK + 1, :])
                rc = smp.tile([1, SP], F32, name="rc")
                nc.vector.reciprocal_approx_fast(rc[:], sm[:])
                bc = smp.tile([64, SP], F32, name="bc")
                nc.gpsimd.partition_broadcast(bc[:], rc[:])
                nc.vector.tensor_tensor(
                    CT[r:r + 64, m, :], cps[0:DK, :], bc[:], MULT)
            return run

        def make_wo_ops(q, CT, split_copies=False):
            # each (si, dh) group split into two single-matmul halves so the
            # interleave never adds more than one extra matmul per period
            ops = []
            for si in range(SI):
                sc = SI * q + si
                for dh in range(D // SP):
                    on_act = split_copies and (si * (D // SP) + dh) % 2 == 1
                    cell = []
                    def half_a(si=si, dh=dh, CT=CT, cell=cell):
                        cell.append(psum.tile([128, SP], F32, tag="wo",
                                              name="po"))
                        nc.tensor.matmul(
                            cell[0][:], CT[:, 0, si * 128:(si + 1) * 128],
                            wo_sb[:, 0, dh * SP:(dh + 1) * SP],
                            start=True, stop=False)
                    def half_b(si=si, sc=sc, dh=dh, CT=CT, cell=cell,
                               on_act=on_act):
                        po = cell[0]
                        nc.tensor.matmul(
                            po[:], CT[:, 1, si * 128:(si + 1) * 128],
                            wo_sb[:, 1, dh * SP:(dh + 1) * SP],
                            start=False, stop=True)
                        ob = obp.tile([128, SP], F32, name="ob")
                        if on_act:
                            nc.scalar.activation(
                                ob[:], po[:],
                                mybir.ActivationFunctionType.Copy)
                        else:
                            nc.vector.tensor_copy(ob[:], po[:])
                        nc.sync.dma_start(
                            out_e.ap()[sc * 128:(sc + 1) * 128,
                                       dh * SP:(dh + 1) * SP], ob[:])
                    ops.append(half_a)
                    ops.append(half_b)
            return ops

        # One flat pipeline across all spans: ctx lags one job, norms
        # run two jobs late, the previous span's W_o and the next span's
        # Q^T projection interleave — nothing flushes at span edges
        # except at the very end of the kernel.
        JPS = HPG * (NSC // 2)          # jobs per span
        QT_t = {0: qpool.tile([128, 2, SP], BF16, name="QT")}
        emit_qproj_group(QT_t[0], 0, 0)
        CT_t = {}
        cps_t = {}
        LAG = 2
        pend_q = []                     # [(q, h, scp, pt)]
        deferred = []                   # (due_gidx, closure)
        wo_queue = []
        # deferred prologue work, one op per early span-0 job, on wo-tag
        # psum slots (no W_o traffic exists yet in span 0)
        fill_queue = [lambda sc=sc: emit_vproj_group(sc, tag="wo")
                      for sc in range(NV_PRE, NSC)]
        fill_queue += [lambda q=q: emit_kproj_group(1, q, tag="wo")
                       for q in range(NQS)]
        fill_queue.append(lambda: emit_qproj_group(QT_t[0], 0, 1))

        for q in range(NQS):
            CT_t[q] = cpool.tile([128, 2, SP], BF16, name="CT")
            cps_t[q] = {}
            qpart_queue = []
            if q + 1 < NQS:
                QT_t[q + 1] = qpool.tile([128, 2, SP], BF16, name="QT")
                qpart_queue = (make_qproj_parts(QT_t[q + 1], q + 1, 0)
                               + make_qproj_parts(QT_t[q + 1], q + 1, 1))

            for jidx in range(JPS):
                gidx = q * JPS + jidx
                h, scp = divmod(jidx, NSC // 2)
                pt = emit_lg_exp(QT_t[q], h, scp)
                pend_q.append((q, h, scp, pt))
                if len(pend_q) > LAG:
                    ent0 = pend_q.pop(0)
                    nrm = emit_ctx(CT_t[ent0[0]], cps_t[ent0[0]], *ent0[1:])
                    if nrm is not None:
                        deferred.append((gidx + 2, nrm))
                for ent in list(deferred):
                    if ent[0] <= gidx:
                        ent[1]()
                        deferred.remove(ent)
                if fill_queue:
                    fill_queue.pop(0)()
                if wo_queue and jidx >= 4 and (jidx - 4) % 3 in (0, 1):
                    wo_queue.pop(0)()
                if qpart_queue and jidx in (13, 15, 21, 23):
                    qpart_queue.pop(0)()
            wo_queue.extend(make_wo_ops(q, CT_t[q],
                                           split_copies=(q == NQS - 1)))

        # epilogue: drain the pipeline
        last_nrm = None
        for ent0 in pend_q:
            nrm = emit_ctx(CT_t[ent0[0]], cps_t[ent0[0]], *ent0[1:])
            if nrm is not None:
                last_nrm = nrm
        for ent in deferred:
            ent[1]()
        if last_nrm is not None:
            last_nrm()
        for op in wo_queue:
            op()

        if debug_dump:
            nc.sync.dma_start(ct_d.ap(), CT_t[NQS - 1][:])
            nc.sync.dma_start(kt_d.ap(), KT_sb[:])
            nc.sync.dma_start(v_d.ap(), V_sb[:])

    nc.compile()
    return nc


_NC_CACHE = {}


def get_nc(S):
    if S not in _NC_CACHE:
        _NC_CACHE[S] = build(S)
    return _NC_CACHE[S]


def make_in_maps(x, W_q, b_q, W_k, b_k, W_v, b_v, W_o, b_o):
    B, S, _ = x.shape
    bf = ml_dtypes.bfloat16
    in_maps = []
    vmask = np.zeros((1, HPG * HAUG), np.float32)
    vmask[0, DK::HAUG] = 1.0
    for core in range(NCORES):
        b, g = divmod(core, GPB)
        sl = slice(HD * g, HD * (g + 1))
        wv_aug = np.zeros((D, HPG * HAUG), np.float32)
        wv_block = np.asarray(W_v[:, sl]).reshape(D, HPG, DK)
        wv_aug.reshape(D, HPG, HAUG)[:, :, :DK] = wv_block
        def wtile(w):
            # [D, N] -> [128, D//128, N] partition-major chunk layout
            return np.ascontiguousarray(
                np.asarray(w).reshape(D // 128, 128, -1).transpose(1, 0, 2))
        in_maps.append({
            "xT": np.ascontiguousarray(
                np.asarray(x[b]).T.reshape(D // 128, 128, S // SP, SP)
                .transpose(2, 1, 0, 3)).astype(bf),
            "wq": wtile(W_q[:, sl]).astype(bf),
            "wk": wtile(W_k[:, sl]).astype(bf),
            "wv": wtile(wv_aug).astype(bf),
            "vmask": vmask.astype(bf),
            "wo": np.ascontiguousarray(
                np.asarray(W_o[sl, :]).reshape(2, 128, D)
                .transpose(1, 0, 2)).astype(bf),
            "bq": np.ascontiguousarray(
                np.asarray(b_q[sl]).reshape(2, 128).T).astype(np.float32),
            "bk": np.ascontiguousarray(
                np.asarray(b_k[sl]).reshape(2, 128).T).astype(np.float32),
            "ones": np.ones((128, HPG), dtype=bf),
        })
    return in_maps


def unshard(results, x, W_o, b_v, b_o):
    B, S, _ = x.shape
    out = np.zeros((B, S, D), np.float32)
    for core in range(NCORES):
        b = core // GPB
        out[b] += results[core]["out"]
    const = np.asarray(b_v).astype(np.float64) @ np.asarray(W_o).astype(np.float64)
    const += np.asarray(b_o).astype(np.float64)
    out += const.astype(np.float32)[None, None, :]
    return out


def run(inputs, trace=False):
    x = np.asarray(inputs["x"])
    nc = get_nc(x.shape[1])
    in_maps = make_in_maps(
        x, inputs["W_q"], inputs["b_q"], inputs["W_k"], inputs["b_k"],
        inputs["W_v"], inputs["b_v"], inputs["W_o"], inputs["b_o"])
    try:
        res = run_bass_kernel_spmd(
            nc, in_maps, core_ids=list(range(NCORES)), trace=trace)
    except Exception:
        # transient device errors (e.g. NRT_EXEC_UNIT_UNRECOVERABLE) clear
        # on re-execution of the same NEFF
        res = run_bass_kernel_spmd(
            nc, in_maps, core_ids=list(range(NCORES)), trace=trace)
    out = unshard(res.results, x, inputs["W_o"], inputs["b_v"], inputs["b_o"])
    return out, res


def kernel(**inputs):
    out, _ = run(inputs, trace=False)
    return out
